# revision 15
# baseline (speedup 1.0000x reference)
"""Trainium2 Bass kernel for nn_NERModel loss (CE + quadruplet + context MSE).

v3 strategy (8 NeuronCores, data-parallel over batch):
  - Host converts embeddings to bf16 -> DMA volume halves (6.3 MB/core).
  - embT (h on partitions, tokens on the free axis) is produced directly by
    the DMA crossbar transpose (dma_start_transpose) while loading from
    DRAM: 4 blocks x 3 h-chunks of [2048 tok, 128 h] -> [128, 2048].
    No PE transposes, no PSUM staging, no SBUF copies.
  - CE: logitsT[17,512] per group of 512 tokens, 4 groups stacked per PSUM
    bank at partition 32*j; exp on ScE per block; per-(group,token) sumexp
    via one row-placement matmul per block into a persistent bank;
    selected-logit sum via mul + tensor_scalar-accumulate against a
    host-built one-hot (tensor_tensor_reduce crashes TRN2 - do not use);
    single Ln + weighted reduces at the end.
  - CTX: adjacent-token diffs are adjacent columns of embT -> one strided
    DVE subtract + one square per group (bf16), then a row-placement
    matmul accumulates per-(group,pair) ||diff||^2 into a persistent
    [16,512] PSUM bank; host-built 0/1 pair weights applied at the end.
    Block-boundary pairs (3/core) are added on host.
  - Device returns two partial sums per core; host does the tiny
    quadruplet term and final combination.
"""

import os
import sys

for _p in ("/opt/trn_rl_repo", "/root/.axon_site/_ro/trn_rl_repo"):
    if _p not in sys.path:
        sys.path.append(_p)

import numpy as np
import ml_dtypes
from contextlib import ExitStack

import concourse.bass as bass
import concourse.bacc as bacc
import concourse.mybir as mybir
from concourse import tile
from concourse.ap import AP

NUM_LABELS = 17
MARGIN = 1.0
IGNORE = -100

B, S, H, L = 64, 1024, 384, NUM_LABELS
NCORES = 8
BP = B // NCORES            # batches per core
NTOK = BP * S               # tokens per core (8192)
NG = 16                     # groups of 512 tokens
NB = 4                      # blocks of 4 groups (2048 tokens)
F32 = mybir.dt.float32
BF16 = mybir.dt.bfloat16
BF = ml_dtypes.bfloat16


def _build_nc() -> bass.Bass:
    # how many groups' squares run on ScE (rest on VE) - rebalance knob
    sq_act = int(os.environ.get("NER_SQ_ACT", "12"))

    nc = bacc.Bacc("TRN2", debug=False)

    emb = nc.declare_dram_parameter("emb", [NTOK, H], BF16, isOutput=False)
    wt = nc.declare_dram_parameter("wt", [128, 3 * L], BF16, isOutput=False)
    selg4 = nc.declare_dram_parameter("selg4", [128, 4], BF16, isOutput=False)
    egall = nc.declare_dram_parameter("egall", [128, NG * NG], BF16, isOutput=False)
    bcol = nc.declare_dram_parameter("bcol", [128, 1], F32, isOutput=False)
    woh = nc.declare_dram_parameter("woh", [128, NB * 512], BF16, isOutput=False)
    cews = nc.declare_dram_parameter("cews", [128, 512], F32, isOutput=False)
    pairw = nc.declare_dram_parameter("pairw", [NG, 512], F32, isOutput=False)
    ones = nc.declare_dram_parameter("ones", [128, 1], F32, isOutput=False)
    outv = nc.declare_dram_parameter("outv", [1, 8], F32, isOutput=True)

    AF = mybir.ActivationFunctionType
    AX = mybir.AxisListType
    OP = mybir.AluOpType

    with tile.TileContext(nc) as tc, ExitStack() as ctx:
        consts = ctx.enter_context(tc.tile_pool(name="consts", bufs=1))
        embt_pool = ctx.enter_context(tc.tile_pool(name="embt", bufs=4))
        d_pool = ctx.enter_context(tc.tile_pool(name="dbuf", bufs=4))
        sq_pool = ctx.enter_context(tc.tile_pool(name="sqbuf", bufs=4))
        expt_pool = ctx.enter_context(tc.tile_pool(name="expt", bufs=2))
        junk_pool = ctx.enter_context(tc.tile_pool(name="junk", bufs=2))
        acc_pool = ctx.enter_context(tc.tile_pool(name="acc", bufs=1))
        ps_l = ctx.enter_context(tc.tile_pool(name="ps_l", bufs=4, space="PSUM"))
        ps_a = ctx.enter_context(tc.tile_pool(name="ps_a", bufs=1, space="PSUM"))
        ps_c = ctx.enter_context(tc.tile_pool(name="ps_c", bufs=1, space="PSUM"))

        embt_blks = {}

        def do_dma(blk: int):
            # xbar-transposed loads go first, on the otherwise-idle sync queue
            embT = embt_pool.tile([128, 3, 2048], BF16, tag="embTblk")
            for c in range(3):
                src = AP(
                    tensor=emb,
                    offset=(blk * 2048) * H + c * 128,
                    ap=[[H, 2048], [1, 128]],
                )
                nc.sync.dma_start_transpose(embT[:, c, :], src)
            embt_blks[blk] = embT

        for blk in range(NB):
            do_dma(blk)

        def cload(handle, shape, dtype):
            # consts ride the scalar hwdge queue so they don't delay the xbar
            t = consts.tile(list(shape), dtype, tag=handle.name + "_c")
            nc.scalar.dma_start(out=t[:], in_=handle.ap())
            return t

        wt_t = cload(wt, (128, 3 * L), BF16)
        bcol_t = cload(bcol, (128, 1), F32)
        selg4_t = cload(selg4, (128, 4), BF16)
        egall_t = cload(egall, (128, NG * NG), BF16)
        woh_t = cload(woh, (128, NB * 512), BF16)
        cews_t = cload(cews, (128, 512), F32)
        pairw_t = cload(pairw, (NG, 512), F32)
        ones_t = cload(ones, (128, 1), F32)

        # persistent accumulators
        bankA = ps_a.tile([128, 512], F32)      # per-(group,token) sumexp
        nc.vector.memset(bankA[:], 1.0)         # ln(1)=0 on unused rows
        ctxps = ps_c.tile([NG, 512], F32)       # per-(group,pair) ||diff||^2
        lg = [
            ps_l.tile([128, 512], F32, tag="lg", name=f"lgbank{i}") for i in range(4)
        ]
        for i in range(4):
            nc.vector.memset(lg[i][:], 0.0)     # exp(0)=1 on unused rows,
        selbuf = acc_pool.tile([128, NB], F32)  # zeroed by selg4/woh

        def do_group(g: int):
            b, j = g // 4, g % 4
            embT = embt_blks[b]
            koff = 512 * j
            w = 512 if j < 3 else 511   # last in-block pair is block-boundary

            # ---- logits into lg[b] rows 32j..32j+16 ----
            lgb = lg[b]
            for c in range(3):
                nc.tensor.matmul(
                    lgb[32 * j : 32 * j + L, :],
                    wt_t[:, c * L : (c + 1) * L],
                    embT[:, c, koff : koff + 512],
                    start=(c == 0), stop=(c == 2),
                    tile_position=(0, 32 * j),
                )

            # ---- ctx: d = embT[:, :, k+1] - embT[:, :, k]; sq = d*d ----
            dt = d_pool.tile([128, 3, 512], BF16, tag="dt")
            nc.vector.tensor_sub(
                dt[:, :, :w], embT[:, :, koff + 1 : koff + 1 + w],
                embT[:, :, koff : koff + w],
            )
            sq = sq_pool.tile([128, 3, 512], BF16, tag="sq")
            if g % NG < sq_act:
                nc.scalar.activation(sq[:, :, :w], dt[:, :, :w], AF.Square)
            else:
                nc.vector.tensor_mul(sq[:, :, :w], dt[:, :, :w], dt[:, :, :w])
            for c in range(3):
                nc.tensor.matmul(
                    ctxps[:, :w],
                    egall_t[:, g * NG : (g + 1) * NG],
                    sq[:, c, :w],
                    start=(g == 0 and c == 0), stop=(g == NG - 1 and c == 2),
                )

            # ---- block postprocess after last group of block ----
            if j == 3:
                ex = expt_pool.tile([128, 512], BF16, tag="ex")
                nc.scalar.activation(
                    ex[:], lgb[:], AF.Exp, bias=bcol_t[:, 0:1], scale=1.0
                )
                nc.tensor.matmul(
                    bankA[32 * b : 32 * b + 4, :], selg4_t[:], ex[:],
                    start=True, stop=True, tile_position=(0, 32 * b),
                )
                jt = junk_pool.tile([128, 512], F32, tag="jt")
                nc.vector.tensor_mul(
                    jt[:], lgb[:], woh_t[:, b * 512 : (b + 1) * 512]
                )
                jt2 = junk_pool.tile([128, 512], F32, tag="jt")
                nc.vector.tensor_scalar(
                    out=jt2[:], in0=jt[:], scalar1=1.0, scalar2=None,
                    op0=OP.mult, op1=OP.add,
                    accum_out=selbuf[:, b : b + 1],
                )

        for g in range(NG):
            do_group(g)

        # ---- final reduction ----
        lnsum = acc_pool.tile([128, 512], F32)
        nc.scalar.activation(lnsum[:], bankA[:], AF.Ln)
        acc1 = acc_pool.tile([128, 1], F32)
        jf1 = junk_pool.tile([128, 512], F32, tag="jt")
        nc.vector.tensor_mul(jf1[:], lnsum[:], cews_t[:])
        jf2 = junk_pool.tile([128, 512], F32, tag="jt")
        nc.vector.tensor_scalar(
            out=jf2[:], in0=jf1[:], scalar1=1.0, scalar2=None,
            op0=OP.mult, op1=OP.add, accum_out=acc1[:, 0:1],
        )
        selsum = acc_pool.tile([128, 1], F32)
        nc.vector.tensor_reduce(selsum[:], selbuf[:], axis=AX.X, op=OP.add)
        cev = acc_pool.tile([128, 1], F32)
        nc.vector.tensor_sub(cev[:], acc1[:], selsum[:])
        fin1 = ps_l.tile([1, 1], F32, tag="lg", name="fin1")
        nc.tensor.matmul(fin1[:], cev[:], ones_t[:], start=True, stop=True)

        junkC = acc_pool.tile([NG, 512], F32)
        acc3 = acc_pool.tile([NG, 1], F32)
        nc.vector.tensor_mul(junkC[:], ctxps[:], pairw_t[:])
        junkD = acc_pool.tile([NG, 512], F32)
        nc.vector.tensor_scalar(
            out=junkD[:], in0=junkC[:], scalar1=1.0, scalar2=None,
            op0=OP.mult, op1=OP.add, accum_out=acc3[:, 0:1],
        )
        fin2 = ps_l.tile([1, 1], F32, tag="lg", name="fin2")
        nc.tensor.matmul(fin2[:], acc3[:], ones_t[0:NG, :], start=True, stop=True)

        outs = acc_pool.tile([1, 8], F32)
        nc.vector.memset(outs[:], 0.0)
        nc.scalar.copy(outs[0:1, 0:1], fin1[:])
        nc.scalar.copy(outs[0:1, 1:2], fin2[:])
        nc.sync.dma_start(out=outv.ap(), in_=outs[:])

    nc.compile()
    return nc


# ---------------------------------------------------------------------------
# host-side preparation


def _host_tables(labf: np.ndarray):
    """Per-core CE/ctx weight tables. labf: [NTOK] int64.

    Row layouts match the device PSUM stacking:
      lg rows 32*j + l  (j = group-within-block, l = label)
      bankA rows 32*b + j  (b = block, j = group-within-block)
      ctxps rows g (group), cols k: pair (512g+k, 512g+k+1)
    """
    valid = labf != IGNORE
    lf = labf.astype(np.int64)
    t = np.arange(NTOK)
    g = t // 512
    k = t % 512
    b_blk = g // 4
    j_grp = g % 4

    woh = np.zeros((128, NB * 512), np.float32)
    lab_c = np.where(valid, lf, 0)
    rows = 32 * j_grp + lab_c
    cols = b_blk * 512 + k
    woh[rows[valid], cols[valid]] = 1.0

    cews = np.zeros((128, 512), np.float32)
    cews[32 * b_blk[valid] + j_grp[valid], k[valid]] = 1.0

    pair_ok = np.zeros(NTOK, dtype=bool)
    kk = np.arange(NTOK - 1)
    in_batch = (kk % S) != (S - 1)
    pair_ok[:-1] = in_batch & (lf[:-1] != IGNORE) & (lf[:-1] == lf[1:]) & (lf[:-1] > 0)
    pairw = np.zeros((NG, 512), np.float32)
    m = np.ones(NTOK, dtype=bool)
    m[-1] = False                       # no pair after last token
    m &= (t % 2048) != 2047             # block-boundary pairs done on host
    pairw[g[m], k[m]] = pair_ok[m].astype(np.float32)

    return woh.astype(BF), cews, pairw


def _quad_host(fe: np.ndarray, fl: np.ndarray, fm: np.ndarray) -> np.float32:
    """Mirror of the reference quadruplet loss in numpy float32."""
    N = fe.shape[0]
    idx = np.arange(N, dtype=np.int64)
    BIG = N
    fm_b = fm > 0
    is_ent = fm_b & (fl > 0)
    non_ent = fm_b & (fl == 0)
    d_i = np.min(np.where(non_ent, idx, BIG))
    has_non = bool(non_ent.any())

    a_i = np.zeros(L - 1, np.int64)
    p_i = np.zeros(L - 1, np.int64)
    n_i = np.zeros(L - 1, np.int64)
    ok = np.zeros(L - 1, bool)
    for i, ty in enumerate(range(1, L)):
        m = is_ent & (fl == ty)
        order = np.sort(np.where(m, idx, BIG))
        a_i[i], p_i[i] = order[0], order[1]
        cnt = int(m.sum())
        other = is_ent & (fl != ty)
        n_i[i] = np.min(np.where(other, idx, BIG))
        ok[i] = (cnt >= 2) and bool(other.any()) and has_non

    clip = lambda v: np.clip(v, 0, N - 1)
    A = fe[clip(a_i)]
    P = fe[clip(p_i)]
    Ng = fe[clip(n_i)]
    D = fe[clip(np.array([d_i]))]
    eps = np.float32(1e-6)

    def dist(x, y):
        d = (x - y + eps).astype(np.float32)
        return np.sqrt(np.sum(d * d, axis=-1, dtype=np.float32)).astype(np.float32)

    pd, nd, dd = dist(A, P), dist(A, Ng), dist(A, D)
    ql = np.maximum(pd - nd + np.float32(MARGIN), 0) + np.maximum(
        pd - dd + np.float32(2.0 * MARGIN), 0
    )
    qcnt = int(ok.sum())
    quad = float(np.sum(np.where(ok, ql, 0.0), dtype=np.float64)) / max(qcnt, 1)
    return np.float32(quad if qcnt > 0 else 0.0)


_NC_CACHE = {}


def _get_nc():
    if "nc" not in _NC_CACHE:
        _NC_CACHE["nc"] = _build_nc()
    return _NC_CACHE["nc"]


def _device_consts():
    if "consts" in _NC_CACHE:
        return _NC_CACHE["consts"]
    selg4 = np.zeros((128, 4), np.float32)
    for j in range(4):
        selg4[32 * j : 32 * j + L, j] = 1.0
    egall = np.zeros((128, NG * NG), np.float32)
    for g in range(NG):
        egall[:, g * NG + g] = 1.0
    ones = np.ones((128, 1), np.float32)
    _NC_CACHE["consts"] = (selg4.astype(BF), egall.astype(BF), ones)
    return _NC_CACHE["consts"]


def _build_in_maps(embeddings, classifier_w, classifier_b, labels):
    emb = np.asarray(embeddings, dtype=np.float32).reshape(B * S, H)
    emb_bf = np.ascontiguousarray(emb).astype(BF)
    W = np.asarray(classifier_w, dtype=np.float32)
    b = np.asarray(classifier_b, dtype=np.float32)
    lab_f = np.asarray(labels).reshape(-1).astype(np.int64)

    wt = np.zeros((128, 3 * L), np.float32)
    for c in range(3):
        wt[:, c * L : (c + 1) * L] = W[:, c * 128 : (c + 1) * 128].T
    bcol = np.zeros((128, 1), np.float32)
    for j in range(4):
        bcol[32 * j : 32 * j + L, 0] = b
    selg4, egall, ones = _device_consts()

    in_maps = []
    for cidx in range(NCORES):
        sl = slice(cidx * NTOK, (cidx + 1) * NTOK)
        woh, cews, pairw = _host_tables(lab_f[sl])
        in_maps.append(
            {
                "emb": emb_bf[sl],
                "wt": wt.astype(BF),
                "selg4": selg4,
                "egall": egall,
                "bcol": bcol,
                "woh": woh,
                "cews": cews,
                "pairw": pairw,
                "ones": ones,
            }
        )
    return in_maps, emb, lab_f, b


def kernel(embeddings, classifier_w, classifier_b, labels, attention_mask):
    from concourse.bass_utils import run_bass_kernel_spmd

    in_maps, emb, lab_f, b = _build_in_maps(
        embeddings, classifier_w, classifier_b, labels
    )
    msk_f = np.asarray(attention_mask).reshape(-1).astype(np.int64)
    N = B * S

    nc = _get_nc()
    res = run_bass_kernel_spmd(nc, in_maps, list(range(NCORES)))

    ce_sum = 0.0
    ctx_sum = 0.0
    for cidx in range(NCORES):
        out = res.results[cidx]["outv"]
        ce_sum += float(out[0, 0])
        ctx_sum += float(out[0, 1])

    valid = lab_f != IGNORE
    ce_cnt = int(valid.sum())
    # device sel used logits without bias; correct with sum(cew * b[label])
    lab_safe = np.where(valid, lab_f, 0)
    ce_sum -= float(np.sum(np.where(valid, b[lab_safe], 0.0), dtype=np.float64))
    ce = ce_sum / max(ce_cnt, 1)

    pair_ok = np.zeros(N, dtype=bool)
    kk = np.arange(N - 1)
    in_batch = (kk % S) != (S - 1)
    pair_ok[:-1] = (
        in_batch & (lab_f[:-1] != IGNORE) & (lab_f[:-1] == lab_f[1:]) & (lab_f[:-1] > 0)
    )
    # block-boundary pairs (t % 2048 == 2047) are not covered on device
    t_bound = np.arange(2047, N - 1, 2048)
    t_bound = t_bound[pair_ok[t_bound]]
    if t_bound.size:
        dif = emb[t_bound + 1] - emb[t_bound]
        ctx_sum += float(np.sum(dif * dif, dtype=np.float64))
    pc = int(pair_ok.sum())
    ctx = (ctx_sum / H) / max(pc, 1) if pc > 0 else 0.0

    quad = _quad_host(emb, lab_f, msk_f)

    loss = ce + 0.5 * float(quad) + 0.1 * ctx
    return np.float32(loss)
